# revision 1
# baseline (speedup 1.0000x reference)
"""Trainium2 Bass kernel for MACE-style message-passing convolution.

Strategy: host sorts edges by receiver and shards them across 8 cores by
receiver node range (2500 nodes/core). Each core gathers sender features
with dma_gather, runs the radial MLP on the PE (f32r/bf16), computes the
tensor product on DVE/ACT/GPSIMD, and segment-sums messages into PSUM via
one-hot matmuls (128-node windows). No collectives needed: core k owns
output rows [2500k, 2500k+2500).
"""
import sys

sys.path.insert(0, "/opt/trn_rl_repo")

import numpy as np
import ml_dtypes

from concourse import bass, bacc, tile, mybir
from concourse.bass_utils import run_bass_kernel_spmd

F32 = mybir.dt.float32
F32R = mybir.dt.float32r
BF16 = mybir.dt.bfloat16
I16 = mybir.dt.int16
AF = mybir.ActivationFunctionType
ALU = mybir.AluOpType

C = 64
N_NODES = 20000
N_EDGES = 320000
RAD = 8
HID = 64
NCORES = 8
NPC = N_NODES // NCORES          # nodes per core = 2500
WIN = 128                        # nodes per psum window
NWIN = (NPC + WIN - 1) // WIN    # 20 windows (last has 68 nodes)
CHUNK = 384                      # edges per MLP matmul chunk (3 tiles)

_cache = {}
ACT_FUNC = AF.Silu


def _prep_inputs(node_feats, vectors, radial_embedding, senders, receivers,
                 W0, W1, W2, W3, K):
    """Build per-core device arrays. K = tiles(128 edges) per 128-node window."""
    EPW = K * 128                # padded edges per window
    EPC = EPW * NWIN             # padded edges per core
    NT = K * NWIN                # tiles per core

    # i-major node feature layout: [s | vx | vy | vz], bf16
    s = node_feats[:, :C]
    v = node_feats[:, C:].reshape(N_NODES, C, 3)
    nf_im = np.concatenate([s, v[:, :, 0], v[:, :, 1], v[:, :, 2]], axis=1)
    nf_bf = nf_im.astype(ml_dtypes.bfloat16)

    # weight folding (see reference._mlp):
    #  h = silu(x @ W0/sqrt(8)); h = silu(h @ W1/8); h = silu(h @ W2/8)
    #  mix = h @ W3/8 ; msgs scaled by mix, out /= 16
    # sign/scale folds: tp_s uses +v_hat (ref has -sqrt(3)*v_hat and /sqrt(3))
    #  -> mix1 cols *(-1); tpv uses a=s_e*mix3', Y=+v_hat -> mix3 cols *(-sqrt(3))
    w0 = (W0 / np.sqrt(8.0)).astype(ml_dtypes.bfloat16)
    w1 = (W1 / 8.0).astype(np.float32)
    w2 = (W2 / 8.0).astype(np.float32)
    w3 = (W3 / 8.0 / 16.0).astype(np.float32).copy()
    w3[:, 64:128] *= -1.0
    w3[:, 192:256] *= -np.sqrt(3.0)
    w1d = np.concatenate([w1, w1], axis=0).astype(ml_dtypes.bfloat16)
    w2d = np.concatenate([w2, w2], axis=0).astype(ml_dtypes.bfloat16)
    w3d = np.concatenate([w3, w3], axis=0).astype(ml_dtypes.bfloat16)

    iota = np.broadcast_to(np.arange(128, dtype=np.float32), (128, 128))
    iota = np.ascontiguousarray(iota).astype(ml_dtypes.bfloat16)

    order = np.argsort(receivers, kind="stable")
    r_sorted = receivers[order]

    in_maps = []
    for k in range(NCORES):
        base = k * NPC
        lo = np.searchsorted(r_sorted, base)
        hi = np.searchsorted(r_sorted, base + NPC)
        eidx = order[lo:hi]          # edges for this core, receiver-sorted
        rk = receivers[eidx] - base  # in [0, NPC)

        sid = np.zeros(EPC, dtype=np.int16)
        rrel = np.full(EPC, -1.0, dtype=np.float32)
        vec = np.zeros((EPC, 3), dtype=np.float32)
        vec[:, 0] = 1.0
        rad = np.zeros((EPC, RAD), dtype=np.float32)

        wstart = np.searchsorted(rk, np.arange(NWIN) * WIN)
        wend = np.searchsorted(rk, np.minimum(np.arange(1, NWIN + 1) * WIN, NPC))
        for w in range(NWIN):
            e = eidx[wstart[w]:wend[w]]
            n = len(e)
            assert n <= EPW, f"window overflow: {n} > {EPW}"
            o = w * EPW
            sid[o:o + n] = senders[e].astype(np.int16)
            rrel[o:o + n] = (receivers[e] - base - w * WIN).astype(np.float32)
            vec[o:o + n] = vectors[e]
            rad[o:o + n] = radial_embedding[e]

        # device layouts
        sidx = np.tile(sid.reshape(-1, 16).T, (8, 1))          # [128, EPC/16]
        vect_t = vec.reshape(NT, 128, 3).transpose(1, 0, 2)    # [128, NT, 3]
        rrel_t = rrel.reshape(NT, 128).T                       # [128, NT]
        radT = np.ascontiguousarray(rad.T).astype(ml_dtypes.bfloat16)  # [8, EPC]

        w0p = np.zeros((128, 64), dtype=ml_dtypes.bfloat16)
        w0p[:8] = w0
        blob = np.concatenate([
            np.ascontiguousarray(w0p).view(np.uint8),
            np.ascontiguousarray(w1d).view(np.uint8),
            np.ascontiguousarray(w2d).view(np.uint8),
            np.ascontiguousarray(w3d).view(np.uint8),
            np.ascontiguousarray(iota).view(np.uint8),
            np.ascontiguousarray(rrel_t).view(np.uint8),
            np.ascontiguousarray(vect_t.reshape(128, -1)).view(np.uint8),
            np.ascontiguousarray(sidx).view(np.uint8),
        ], axis=1)

        in_maps.append({
            "nf": nf_bf,
            "radT": radT,
            "consts": blob,
        })
    return in_maps


def _build_program(K):
    EPW = K * 128
    NT = K * NWIN
    nc = bacc.Bacc()

    nf_d = nc.dram_tensor("nf", [N_NODES, 256], BF16, kind="ExternalInput")
    radT_d = nc.dram_tensor("radT", [8, NWIN * EPW], BF16, kind="ExternalInput")
    CB = 128 + 128 + 128 + 512 + 256 + NT * 4 + NT * 12 + NT * 16  # bytes/row
    consts_d = nc.dram_tensor("consts", [128, CB], mybir.dt.uint8,
                              kind="ExternalInput")
    out_d = nc.dram_tensor("out", [NPC, 512], F32, kind="ExternalOutput")

    NCH = EPW // CHUNK  # MLP chunks per window (must divide evenly)
    assert NCH * CHUNK == EPW and NCH % 2 == 0

    with tile.TileContext(nc) as tc:
        with (
            tc.tile_pool(name="const", bufs=1) as cpool,
            tc.tile_pool(name="io", bufs=2) as iopool,
            tc.tile_pool(name="work", bufs=2) as wpool,
            tc.tile_pool(name="psum_mlp", bufs=3, space="PSUM") as pmlp,
            tc.tile_pool(name="psum_mix", bufs=2, space="PSUM") as pmix,
            tc.tile_pool(name="psum_out", bufs=2, space="PSUM") as pout,
        ):
            # ---- preload constants (single DMA -> single wait lane) ----
            blob = cpool.tile([128, CB], mybir.dt.uint8, tag="blob")
            nc.sync.dma_start(blob[:], consts_d[:])
            o = 0
            w0_sb = blob[0:8, o:o + 128].bitcast(BF16); o += 128
            w1_sb = blob[:, o:o + 128].bitcast(BF16); o += 128
            w2_sb = blob[:, o:o + 128].bitcast(BF16); o += 128
            w3_sb = blob[:, o:o + 512].bitcast(BF16); o += 512
            iota_sb = blob[:, o:o + 256].bitcast(BF16); o += 256
            rrel_sb = blob[:, o:o + NT * 4].bitcast(F32); o += NT * 4
            vect_sb = blob[:, o:o + NT * 12].bitcast(F32)
            vect_sb = vect_sb.rearrange("p (t i) -> p t i", t=NT, i=3); o += NT * 12
            sidx_sb = blob[:, o:o + NT * 16].bitcast(I16); o += NT * 16
            assert o == CB

            for w in range(NWIN):
                # ---- loads ----
                gath = iopool.tile([128, K, 256], BF16, tag="gath")
                GC = 768  # idxs per dma_gather call (SWDGE ring capacity)
                for g0 in range(0, EPW, GC):
                    gt = g0 // 128
                    nc.gpsimd.dma_gather(
                        out_ap=gath[:, gt:gt + GC // 128, :],
                        in_ap=nf_d[:],
                        idxs_ap=sidx_sb[:, (w * EPW + g0) // 16:(w * EPW + g0 + GC) // 16],
                        num_idxs=GC,
                        num_idxs_reg=GC,
                        elem_size=256,
                    )
                radT = iopool.tile([8, EPW], BF16, tag="radT")
                nc.sync.dma_start(radT[:], radT_d[:, w * EPW:(w + 1) * EPW])

                # ---- radial MLP on PE ----
                h1 = wpool.tile([128, NCH // 2, CHUNK], BF16, tag="h1")
                h2 = wpool.tile([128, NCH // 2, CHUNK], BF16, tag="h2")
                h3 = wpool.tile([128, NCH // 2, CHUNK], BF16, tag="h3")
                for p in range(NCH // 2):
                    ph = pmlp.tile([128, CHUNK], F32, tag="pmlp")
                    for half in range(2):
                        c = 2 * p + half
                        b = 64 * half
                        nc.tensor.matmul(
                            ph[b:b + 64, :],
                            w0_sb[:],
                            radT[:, c * CHUNK:(c + 1) * CHUNK],
                            start=True, stop=True,
                        )
                    nc.scalar.activation(h1[:, p, :], ph[:], ACT_FUNC)
                    ph2 = pmlp.tile([128, CHUNK], F32, tag="pmlp")
                    for half in range(2):
                        b = 64 * half
                        nc.tensor.matmul(
                            ph2[b:b + 64, :], w1_sb[b:b + 64, :],
                            h1[b:b + 64, p, :], start=True, stop=True,
                        )
                    nc.scalar.activation(h2[:, p, :], ph2[:], ACT_FUNC)
                    ph3 = pmlp.tile([128, CHUNK], F32, tag="pmlp")
                    for half in range(2):
                        b = 64 * half
                        nc.tensor.matmul(
                            ph3[b:b + 64, :], w2_sb[b:b + 64, :],
                            h2[b:b + 64, p, :], start=True, stop=True,
                        )
                    nc.scalar.activation(h3[:, p, :], ph3[:], ACT_FUNC)

                mix = wpool.tile([128, K, 256], BF16, tag="mix")
                for t in range(K):
                    c = (t * 128) // CHUNK
                    off = (t * 128) % CHUNK
                    b = 64 * (c % 2)
                    pm = pmix.tile([128, 256], F32, tag="pm")
                    nc.tensor.matmul(
                        pm[:], h3[b:b + 64, c // 2, off:off + 128],
                        w3_sb[b:b + 64, :], start=True, stop=True,
                    )
                    nc.scalar.activation(mix[:, t, :], pm[:], AF.Copy)

                # ---- Y = v / ||v|| (sign/sqrt3 folded into mix cols) ----
                touch = wpool.tile([128, 1], BF16, tag="touch")
                nc.vector.tensor_tensor(touch[:], gath[:, 0, 0:1], gath[:, 0, 0:1],
                                        ALU.mult)
                vw = vect_sb[:, w * K:(w + 1) * K, :]
                sq = wpool.tile([128, K, 3], F32, tag="sq")
                nc.vector.tensor_tensor(sq[:], vw, vw, ALU.mult)
                ss = wpool.tile([128, K], F32, tag="ss")
                nc.vector.tensor_reduce(ss[:], sq[:], mybir.AxisListType.X, ALU.add)
                rinv = wpool.tile([128, K], F32, tag="rinv")
                nc.vector.reciprocal(rinv[:], ss[:])
                rs = wpool.tile([128, K], F32, tag="rs")
                nc.scalar.activation(rs[:], rinv[:], AF.Sqrt)
                Y = wpool.tile([128, K, 3], F32, tag="Y")
                nc.vector.tensor_tensor(
                    Y[:], vw, rs[:].unsqueeze(-1).broadcast_to([128, K, 3]), ALU.mult)

                # ---- tensor product + mix gating ----
                msg = wpool.tile([128, K, 512], BF16, tag="msg")
                g_s = gath[:, :, 0:64]
                # sem = s_e * mix0
                nc.vector.tensor_tensor(msg[:, :, 0:64], g_s, mix[:, :, 0:64], ALU.mult)
                # vem = v_e * mix2 (i-major, mix2 broadcast over i)
                mix2b = mix[:, :, 128:192].unsqueeze(2).broadcast_to([128, K, 3, 64])
                nc.vector.tensor_tensor(
                    msg[:, :, 128:320].rearrange("p k (i c) -> p k i c", i=3, c=64),
                    gath[:, :, 64:256].rearrange("p k (i c) -> p k i c", i=3, c=64),
                    mix2b, ALU.mult)
                # tp_s = sum_i v_i * Y_i ; tpsm = tp_s * mix1
                pa = wpool.tile([128, K, 64], BF16, tag="pa")
                pb = wpool.tile([128, K, 64], BF16, tag="pb")
                tps = wpool.tile([128, K, 64], BF16, tag="tps")
                yb = [Y[:, :, i:i + 1].broadcast_to([128, K, 64]) for i in range(3)]
                nc.vector.tensor_tensor(pa[:], gath[:, :, 64:128], yb[0], ALU.mult)
                nc.vector.tensor_tensor(pb[:], gath[:, :, 128:192], yb[1], ALU.mult)
                nc.vector.tensor_tensor(pa[:], pa[:], pb[:], ALU.add)
                nc.vector.tensor_tensor(pb[:], gath[:, :, 192:256], yb[2], ALU.mult)
                nc.vector.tensor_tensor(tps[:], pa[:], pb[:], ALU.add)
                nc.vector.tensor_tensor(msg[:, :, 64:128], tps[:], mix[:, :, 64:128], ALU.mult)
                # a = s_e * mix3 ; tpv_i = a * Y_i  (gpsimd)
                av = wpool.tile([128, K, 64], BF16, tag="av")
                nc.vector.tensor_tensor(av[:], g_s, mix[:, :, 192:256], ALU.mult)
                for i in range(3):
                    nc.vector.tensor_tensor(
                        msg[:, :, 320 + 64 * i:384 + 64 * i], av[:],
                        Y[:, :, i:i + 1].broadcast_to([128, K, 64]), ALU.mult)

                # ---- one-hot + scatter matmul ----
                R = wpool.tile([128, K, 128], BF16, tag="R")
                nc.vector.tensor_tensor(
                    R[:], iota_sb[:].unsqueeze(1).broadcast_to([128, K, 128]),
                    rrel_sb[:, w * K:(w + 1) * K].unsqueeze(-1).broadcast_to([128, K, 128]),
                    ALU.is_equal)
                po = pout.tile([128, 512], F32, tag="po")
                for t in range(K):
                    nc.tensor.matmul(po[:], R[:, t, :], msg[:, t, :],
                                     start=(t == 0), stop=(t == K - 1))

                # ---- permute (c,i) and store ----
                osb = iopool.tile([128, 512], F32, tag="osb")
                nc.scalar.activation(osb[:, 0:128], po[:, 0:128], AF.Copy)
                nc.scalar.activation(
                    osb[:, 128:512].rearrange("p (b c i) -> p b i c", b=2, c=64, i=3),
                    po[:, 128:512].rearrange("p (b i c) -> p b i c", b=2, i=3, c=64),
                    AF.Copy)
                rows = min(WIN, NPC - w * WIN)
                nc.sync.dma_start(out_d[w * WIN:w * WIN + rows, :], osb[:rows, :])

    nc.compile()
    return nc


def kernel(node_feats, vectors, radial_embedding, senders, receivers,
           W0, W1, W2, W3):
    node_feats = np.asarray(node_feats, dtype=np.float32)
    vectors = np.asarray(vectors, dtype=np.float32)
    radial_embedding = np.asarray(radial_embedding, dtype=np.float32)
    senders = np.asarray(senders, dtype=np.int32)
    receivers = np.asarray(receivers, dtype=np.int32)

    # K = max tiles needed by any (core, window)
    counts = np.bincount(receivers // WIN if NPC % WIN == 0 else
                         (receivers // NPC) * NWIN + (receivers % NPC) // WIN,
                         minlength=NCORES * NWIN)
    K = int(np.ceil(counts.max() / 128))
    K = ((K + 5) // 6) * 6  # multiple of 6 so CHUNK=384 tiles evenly
    in_maps = _prep_inputs(node_feats, vectors, radial_embedding, senders,
                           receivers, np.asarray(W0, np.float32),
                           np.asarray(W1, np.float32), np.asarray(W2, np.float32),
                           np.asarray(W3, np.float32), K)

    if K not in _cache:
        _cache[K] = _build_program(K)
    nc = _cache[K]

    res = run_bass_kernel_spmd(nc, in_maps, core_ids=list(range(NCORES)))
    out = np.concatenate([res.results[k]["out"] for k in range(NCORES)], axis=0)
    return out.astype(np.float32)


if __name__ == "__main__":
    sys.path.insert(0, "/root/problem")
    import reference
    inputs = {k: np.asarray(v) for k, v in reference.setup_inputs().items()}
    exp = np.asarray(reference.reference(**inputs))
    act = kernel(**inputs)
    err = np.abs(act - exp).max() / (np.abs(exp).max() + 1e-9)
    print("Relative error:", err)

